# revision 4
# baseline (speedup 1.0000x reference)
"""Trainium2 Bass kernel for nn_ClusterLoss (N=4096, D=2048, 8 NeuronCores).

Math (constants ALPHA=6, BETA=2, ANN_R=3, ANN_RR=5, TVAL=1, EPS=1e-5):
  dm = 1 - dist <= 1 < BETA  =>  loss_ap == 0 identically.
  dm < ALPHA always          =>  an_mask == neg (upper-tri & label mismatch).
  loss_an_i = sum_j (5+u_ij) e^(5+u_ij) / (sum_j e^(5+u_ij) + EPS),  u = dist.
Device computes per-row S0 = sum w and S1 = sum u*w with w = e^(u+5) masked;
host does the division, mean, and the annulus term (O(N) work).

Sharding: rows are split into 8 blocks of 512; core c owns the 64-row slice c
of every block ("half-tiles"), pairing blocks (0,1),(2,3),(4,5),(6,7) into 4
fused 128-row m-tiles so the upper-triangular tile skip is load-balanced AND
the program is identical on all cores (SPMD) — only the gathered input data
differs per core.

The [128,512] distance tile comes out of one augmented bf16 matmul:
  lhsT rows 0..2047 = -2*cf_mine.T, then [1, 1, sqh_i, sql_i]
  rhs  rows 0..2047 =    cf_all.T,  then [sqh_j, sql_j, 1, 1]
so PSUM = sq_i + sq_j - 2*cf_i.cf_j exactly (sq split hi+lo in bf16).
A second tiny matmul with +/- one-hot label rows yields (1 - same_label)
directly in PSUM. DVE tensor_mul + reduce_sum do masking and row-sums
(tensor_tensor_reduce faults on this hardware path; plain ops do not).
"""

import sys

sys.path.insert(0, "/opt/trn_rl_repo")

import numpy as np
import ml_dtypes

import concourse.bass as bass
import concourse.mybir as mybir
import concourse.tile as tile
from concourse import bacc
from concourse.bass_utils import run_bass_kernel_spmd

BF16 = ml_dtypes.bfloat16
N, D, NCORES = 4096, 2048, 8
QBLK = 512          # row block per q
HALF = 64           # per-core slice of each q block
KTOT = D + 4        # 2052 augmented K rows
KCH = 17            # ceil(2052/128); padded to 17*128 = 2176 with zeros
KPAD = KCH * 128
NB = 8              # 512-wide n blocks
FT = 4              # fused m-tiles per core

_prog_cache = {}


def _build_program():
    nc = bacc.Bacc("TRN2", target_bir_lowering=False, debug=False,
                   num_devices=NCORES)

    # const AP for the Exp bias (+5.0), registered in the preamble like
    # Bass.__init__ does for 0.0/1.0
    t5 = nc.alloc_sbuf_tensor("const-float32-5.0", [128, 1], mybir.dt.float32)
    nc.gpsimd.memset(t5.ap(), 5.0)
    nc.const_aps.aps[(mybir.dt.float32, 5.0)] = t5.ap()
    nc.all_engine_barrier()

    a_d = nc.dram_tensor("a", [128, NB, KCH, 512], mybir.dt.bfloat16,
                         kind="ExternalInput")
    rm2_d = nc.dram_tensor("rm2", [128, KCH, 512], mybir.dt.bfloat16,
                           kind="ExternalInput")
    oha_d = nc.dram_tensor("oha", [128, N], mybir.dt.bfloat16,
                           kind="ExternalInput")
    ohm_d = nc.dram_tensor("ohm", [128, 512], mybir.dt.bfloat16,
                           kind="ExternalInput")
    mask_d = nc.dram_tensor("masks", [128, NB, 512], mybir.dt.bfloat16,
                            kind="ExternalInput")
    s01_d = nc.dram_tensor("s01", [128, 512], mybir.dt.float32,
                           kind="ExternalOutput")

    fp32 = mybir.dt.float32
    bf16 = mybir.dt.bfloat16

    with tile.TileContext(nc) as tc:
        with (
            tc.tile_pool(name="big", bufs=1) as big,
            tc.tile_pool(name="abuf", bufs=3) as abuf,
            tc.tile_pool(name="acc", bufs=1) as accp,
            tc.tile_pool(name="work", bufs=3) as work,
            tc.tile_pool(name="psum", bufs=2, space="PSUM") as psum,
        ):
            rm2 = big.tile([128, KCH, 512], bf16)
            nc.sync.dma_start(out=rm2[:], in_=rm2_d.ap())
            ohm = big.tile([128, 512], bf16)
            nc.sync.dma_start(out=ohm[:], in_=ohm_d.ap())
            oha = big.tile([128, N], bf16)
            nc.sync.dma_start(out=oha[:], in_=oha_d.ap())
            masks = big.tile([128, NB, 512], bf16)
            nc.sync.dma_start(out=masks[:], in_=mask_d.ap())


            s0col = [accp.tile([128, NB], fp32, tag=f"s0c{f}", name=f"s0c{f}")
                     for f in range(FT)]
            s1col = [accp.tile([128, NB], fp32, tag=f"s1c{f}", name=f"s1c{f}")
                     for f in range(FT)]

            # n-blocks big-to-small so PE stays ahead of the A DMA stream
            for b in range(NB - 1, -1, -1):
                asb = abuf.tile([128, KCH, 512], bf16, tag="asb", name=f"asb{b}")
                nc.sync.dma_start(out=asb[:], in_=a_d.ap()[:, b])
                for f in range(FT):
                    if b < 2 * f:
                        continue  # tile entirely below the diagonal
                    d2 = psum.tile([128, 512], fp32, tag="d2")
                    for k in range(KCH):
                        nc.tensor.matmul(
                            d2[:],
                            rm2[:, k, 128 * f:128 * (f + 1)],
                            asb[:, k],
                            start=(k == 0),
                            stop=(k == KCH - 1),
                        )
                    nm = psum.tile([128, 512], fp32, tag="nm")
                    nc.tensor.matmul(
                        nm[:],
                        ohm[:, 128 * f:128 * (f + 1)],
                        oha[:, 512 * b:512 * (b + 1)],
                        start=True,
                        stop=True,
                    )
                    d2c = work.tile([128, 512], fp32, tag="d2c")
                    nc.vector.tensor_scalar_max(d2c[:], d2[:], 1e-12)
                    u = work.tile([128, 512], fp32, tag="u")
                    nc.scalar.activation(u[:], d2c[:],
                                         mybir.ActivationFunctionType.Sqrt)
                    e = work.tile([128, 512], fp32, tag="e")
                    nc.scalar.activation(e[:], u[:],
                                         mybir.ActivationFunctionType.Exp,
                                         bias=5.0, scale=1.0)
                    if b <= 2 * f + 1:
                        # diagonal-adjacent tile: apply triangle mask to nm
                        nmt = work.tile([128, 512], fp32, tag="nmt")
                        nc.vector.tensor_mul(nmt[:], nm[:], masks[:, b])
                        nm_in = nmt
                    else:
                        nm_in = nm
                    w = work.tile([128, 512], fp32, tag="w")
                    cb = b - 2 * f
                    nc.vector.tensor_mul(w[:], e[:], nm_in[:])
                    nc.vector.reduce_sum(out=s0col[f][:, cb:cb + 1], in_=w[:],
                                         axis=mybir.AxisListType.X)
                    p = work.tile([128, 512], fp32, tag="d2c")
                    nc.vector.tensor_mul(p[:], u[:], w[:])
                    nc.vector.reduce_sum(out=s1col[f][:, cb:cb + 1], in_=p[:],
                                         axis=mybir.AxisListType.X)

            s01 = accp.tile([128, 512], fp32)
            nc.scalar.mul(s01[:], s01[:], 0.0)
            for f in range(FT):
                cnt = NB - 2 * f
                nc.vector.reduce_sum(out=s01[:, f:f + 1], in_=s0col[f][:, :cnt],
                                     axis=mybir.AxisListType.X)
                nc.vector.reduce_sum(out=s01[:, FT + f:FT + f + 1],
                                     in_=s1col[f][:, :cnt],
                                     axis=mybir.AxisListType.X)
            nc.sync.dma_start(out=s01_d.ap(), in_=s01[:])

    nc.compile()
    return nc


def _core_rows(c):
    # column m = 128*f + p  ->  global row 512*(2f + (p>=64)) + 64*c + (p%64)
    f = np.arange(FT)[:, None]
    p = np.arange(128)[None, :]
    q = 2 * f + (p >= 64)
    return (QBLK * q + HALF * c + (p % 64)).reshape(-1)


def kernel(feat, center, labels):
    feat = np.asarray(feat, np.float32)
    center = np.asarray(center, np.float32)
    labels = np.asarray(labels).astype(np.int64)

    cf = feat - center                                   # [N, D] fp32
    sq64 = np.sum(cf.astype(np.float64) ** 2, axis=1)
    sq32 = sq64.astype(np.float32)
    cfb = cf.astype(BF16)
    sqh = sq32.astype(BF16)
    sql = (sq32 - sqh.astype(np.float32)).astype(BF16)

    # shared rhs A [KPAD, N] -> dram layout [128, NB, KCH, 512]
    A = np.zeros((KPAD, N), BF16)
    A[:D] = cfb.T
    A[D] = sqh
    A[D + 1] = sql
    A[D + 2] = np.ones(N, BF16)
    A[D + 3] = np.ones(N, BF16)
    a_dev = np.ascontiguousarray(
        A.reshape(KCH, 128, NB, 512).transpose(1, 2, 0, 3))

    oha = np.zeros((128, N), BF16)
    oh = (labels[None, :] == np.arange(64)[:, None])
    oha[:64] = oh.astype(BF16)
    oha[64] = np.ones(N, BF16)

    if "nc" not in _prog_cache:
        _prog_cache["nc"] = _build_program()
    nc = _prog_cache["nc"]

    in_maps = []
    rows_all = []
    for c in range(NCORES):
        rows = _core_rows(c)
        rows_all.append(rows)
        R = np.zeros((KPAD, 512), BF16)
        R[:D] = (-2.0 * cfb[rows].astype(np.float32)).astype(BF16).T
        R[D] = np.ones(512, BF16)
        R[D + 1] = np.ones(512, BF16)
        R[D + 2] = sqh[rows]
        R[D + 3] = sql[rows]
        rm2_dev = np.ascontiguousarray(
            R.reshape(KCH, 128, 512).transpose(1, 0, 2))

        ohm = np.zeros((128, 512), BF16)
        ohm[:64] = -(labels[rows][None, :] == np.arange(64)[:, None]).astype(BF16)
        ohm[64] = np.ones(512, BF16)

        m = np.zeros((128, NB, 512), BF16)
        jg = np.arange(512)
        for b in range(NB):
            ig = rows[128 * (b // 2):128 * (b // 2) + 128]
            m[:, b, :] = ((512 * b + jg)[None, :] > ig[:, None]).astype(BF16)

        in_maps.append({"a": a_dev, "rm2": rm2_dev, "oha": oha,
                        "ohm": ohm, "masks": m})

    global _last_in_maps
    _last_in_maps = in_maps
    res = run_bass_kernel_spmd(nc, in_maps, list(range(NCORES)))

    S0 = np.zeros(N, np.float32)
    S1 = np.zeros(N, np.float32)
    for c in range(NCORES):
        s01 = np.asarray(res.results[c]["s01"], np.float32)[:, :8]
        S0[rows_all[c]] = s01[:, :FT].T.reshape(-1)
        S1[rows_all[c]] = s01[:, FT:].T.reshape(-1)

    loss_an = (np.float32(5.0) * S0 + S1) / (S0 + np.float32(1e-5))
    ranked = np.mean(loss_an, dtype=np.float32)

    ac = np.sqrt(np.clip(sq64, 1e-12, None))
    under = np.sum(np.where(ac < 3.0, 3.0 - ac, 0.0))
    beyond = np.sum(np.where(ac > 5.0, ac - 5.0, 0.0))
    annulus = np.float32((under + beyond) / N)

    return np.array(ranked + annulus, dtype=np.float32)
